# revision 3
# baseline (speedup 1.0000x reference)
"""Trainium2 kernel for: LayerNorm(d=1024) -> Linear(1024->4096) -> *scale -> 3*tanh(x/3).

Sharding: data-parallel over the batch dim (8 batches -> 8 NeuronCores).
Each core processes one [2048, 1024] shard and the full weight matrix.

Host-side algebraic folding (all O(d_z * d_model), batch-independent):
    y = (LN(z; gamma, beta) @ W + b) * scale
      = ((zhat * gamma + beta) @ W + b) * scale          with zhat = (z - mu) * rstd
      = zhat @ [gamma[:,None] * W * scale] + [(beta @ W + b) * scale]
    out = 3 * tanh(y / 3) = 3 * tanh(zhat @ W2 + b2)     with the /3 folded into W2, b2.

Device per core (per 128-token tile, 16 tiles):
    bn_stats/bn_aggr -> mean/var; sqrt+reciprocal -> rstd   (DVE/ACT)
    zhat = (z - mu) * rstd, cast to bf16                    (DVE, one pass)
    transpose zhat 128x128 chunks via DMA XBAR              (DMA)
    psum = ones/128 @ bias_bcast + sum_k zhatT_k @ W2_k     (PE, bf16, N=512)
    out = tanh(psum) in bf16                                (ACT)
Host: out_f32 = 3 * out_bf16.
"""

import numpy as np
import ml_dtypes

import concourse.bass as bass
import concourse.mybir as mybir
import concourse.tile as tile
from concourse import bacc
from concourse.bass_utils import run_bass_kernel_spmd

N_CORES = 8
TOK = 2048
D_Z = 1024
D_MODEL = 4096
P = 128
K_CHUNKS = D_Z // P        # 8
TOK_TILES = TOK // P       # 16
N_TILE = 512
N_TILES = D_MODEL // N_TILE  # 8
EPS = 1e-5
CLAMP = 3.0

BF16 = mybir.dt.bfloat16
F32 = mybir.dt.float32

_compiled = {}


def _build(TOK=TOK, TOK_TILES=TOK_TILES):
    nc = bacc.Bacc("TRN2", target_bir_lowering=False, debug=False, num_devices=N_CORES)

    z_d = nc.dram_tensor("z", [TOK, D_Z], F32, kind="ExternalInput")
    w_d = nc.dram_tensor("w", [D_Z, D_MODEL], BF16, kind="ExternalInput")
    b_d = nc.dram_tensor("b", [D_MODEL], BF16, kind="ExternalInput")
    out_d = nc.dram_tensor("out", [TOK, D_MODEL], BF16, kind="ExternalOutput")

    with tile.TileContext(nc) as tc:
        with (
            tc.tile_pool(name="singles", bufs=1) as singles,
            tc.tile_pool(name="zpool", bufs=3) as zpool,
            tc.tile_pool(name="znpool", bufs=3) as znpool,
            tc.tile_pool(name="ztpool", bufs=3) as ztpool,
            tc.tile_pool(name="stats", bufs=6) as stats,
            tc.tile_pool(name="opool", bufs=3) as opool,
            tc.tile_pool(name="psum", bufs=4, space="PSUM") as psum_pool,
        ):
            # Weights in SBUF: [128, k_chunk, d_model], loaded in k-chunk pieces
            # so the transfers spread across DMA queues and overlap LN startup.
            w_sb = singles.tile([P, K_CHUNKS, D_MODEL], BF16)
            w_ap = w_d.ap().rearrange("(ko p) m -> p ko m", p=P)
            for ko in range(K_CHUNKS):
                nc.sync.dma_start(out=w_sb[:, ko, :], in_=w_ap[:, ko, :])

            # Bias broadcast to all 128 partitions (partition-step-0 DMA).
            bias_sb = singles.tile([P, D_MODEL], BF16)
            b_ap = b_d.ap()
            b_bcast = bass.AP(
                tensor=b_ap.tensor, offset=b_ap.offset, ap=[[0, P]] + list(b_ap.ap)
            )
            nc.sync.dma_start(out=bias_sb, in_=b_bcast)

            # (1/128) * ones, stationary operand of the bias-init matmul:
            # psum = onesT.T @ bias_bcast = bias row replicated on all partitions.
            ones_sb = singles.tile([P, P], BF16)
            nc.vector.memset(ones_sb, 1.0 / P)

            eps_sb = singles.tile([P, 1], F32)
            nc.vector.memset(eps_sb, EPS)

            z_ap = z_d.ap().rearrange("(t p) d -> t p d", p=P)
            out_ap = out_d.ap().rearrange("(t p) m -> t p m", p=P)

            for t in range(TOK_TILES):
                z_t = zpool.tile([P, D_Z], F32)
                nc.sync.dma_start(out=z_t, in_=z_ap[t])

                # mean/var over the last dim via bn_stats (512-wide subgroups).
                st = stats.tile([P, 2, 6], F32)
                for sg in range(2):
                    nc.vector.bn_stats(
                        out=st[:, sg, :], in_=z_t[:, sg * 512 : (sg + 1) * 512]
                    )
                mv = stats.tile([P, 2], F32)
                nc.vector.bn_aggr(out=mv, in_=st)

                # rstd = 1/sqrt(var + eps)
                rstd = stats.tile([P, 1], F32)
                nc.scalar.activation(
                    out=rstd,
                    in_=mv[:, 1:2],
                    func=mybir.ActivationFunctionType.Sqrt,
                    bias=eps_sb,
                    scale=1.0,
                )
                nc.vector.reciprocal(out=rstd, in_=rstd)

                # zhat = (z - mean) * rstd, cast to bf16 in one DVE pass.
                zn = znpool.tile([P, D_Z], BF16)
                nc.vector.tensor_scalar(
                    out=zn,
                    in0=z_t,
                    scalar1=mv[:, 0:1],
                    scalar2=rstd,
                    op0=mybir.AluOpType.subtract,
                    op1=mybir.AluOpType.mult,
                )

                # Transpose each 128x128 chunk: znt[p_dz, k, tok] = zn[tok, k*128+p_dz]
                znt = ztpool.tile([P, K_CHUNKS, P], BF16)
                for k in range(K_CHUNKS):
                    nc.sync.dma_start(
                        out=znt[:, k, :], in_=zn[:, k * P : (k + 1) * P], transpose=True
                    )

                o_t = opool.tile([P, D_MODEL], BF16)
                for n in range(N_TILES):
                    ns = slice(n * N_TILE, (n + 1) * N_TILE)
                    ps = psum_pool.tile([P, N_TILE], F32)
                    # bias init: psum = sum_k (1/128) * bias_bcast = bias row
                    nc.tensor.matmul(
                        ps, lhsT=ones_sb, rhs=bias_sb[:, ns], start=True, stop=False
                    )
                    for k in range(K_CHUNKS):
                        nc.tensor.matmul(
                            ps,
                            lhsT=znt[:, k, :],
                            rhs=w_sb[:, k, ns],
                            start=False,
                            stop=(k == K_CHUNKS - 1),
                        )
                    nc.scalar.activation(
                        out=o_t[:, ns], in_=ps, func=mybir.ActivationFunctionType.Tanh
                    )
                nc.sync.dma_start(out=out_ap[t], in_=o_t)

    nc.compile()
    return nc


def kernel(z, ln_gamma, ln_beta, W, b, scale):
    if "nc" not in _compiled:
        _compiled["nc"] = _build()
    nc = _compiled["nc"]

    s = float(np.asarray(scale).reshape(-1)[0]) / CLAMP
    w2 = (W.astype(np.float64) * ln_gamma.astype(np.float64)[:, None] * s).astype(
        ml_dtypes.bfloat16
    )
    b2 = ((ln_beta.astype(np.float64) @ W.astype(np.float64) + b) * s).astype(
        ml_dtypes.bfloat16
    )

    z = np.ascontiguousarray(z, dtype=np.float32)
    in_maps = [
        {"z": z[i].reshape(TOK, D_Z), "w": w2, "b": b2} for i in range(N_CORES)
    ]
    res = run_bass_kernel_spmd(nc, in_maps, core_ids=list(range(N_CORES)))

    out = np.empty((N_CORES, TOK, D_MODEL), dtype=np.float32)
    for i in range(N_CORES):
        out[i] = res.results[i]["out"].astype(np.float32)
    out *= CLAMP
    return out
